# revision 16
# baseline (speedup 1.0000x reference)
"""Trainium2 Bass kernel for nn_CognitiveWorkspaceTransformer.

Math (reference semantics):
    X   = S + concat(w_spoke, w_hub_priv, w_hub_shared, tag)   # full 1088 cover
    out = X @ W_read.T          # (B,T,1024)
    k   = latent @ Wk.T         # cache is fully overwritten by latent
    v   = latent @ Wv.T

Sharding: data-parallel over batch B=8, one batch element per NeuronCore.
All tensors are laid out feature-major on the host (pure layout prep plus a
bf16 downcast, no arithmetic) so the contraction dim lands on SBUF
partitions directly and the PE needs no on-chip transposes.

bf16 everywhere (tolerance is 2e-2; bf16 lands ~5e-3): ~47MB/core HBM
traffic (~131us roofline) vs ~360k PE cycles (~150us @ 2.4GHz) -> the PE
array is the bottleneck; everything else is scheduled to keep it fed:
  - a few warm-up junk matmuls at t=0 so the HAM clock gate reaches
    2.4GHz before real work lands (cold matmuls run at 1.2GHz)
  - the ramp is ordered for earliest PE start: Wk/Wv (0.5MB) and the
    first 768 latent columns load first, then slab-0/1 S+wc, then the
    bulky W_read; slabs 0-1 are small (256/512) and run their k/v
    matmuls before the first out-matmul; their k/v stores are deferred
    to slabs 2-3 so ramp loads keep the full HBM bandwidth
  - adds are emitted one slab ahead so they never compete with a slab
    tail's PSUM->SBUF copies on the DVE (pout ring would stall the PE)
  - out-copies alternate DVE/ACT per group pair for the same reason
  - slab i+1 loads are issued BEFORE slab i stores: a store waiting on
    its tile would otherwise block later load issues (in-order queues)
  - j-outer/h-inner so each 128x128 stationary X^T chunk is loaded once
  - 2-bank PSUM tiles [128,1024]; ONE wide PSUM->SBUF cast-copy per
    out/k/v tile; paired [256,1024] stores
"""

import numpy as np
import ml_dtypes

import concourse.bacc as bacc
import concourse.mybir as mybir
import concourse.tile as tile
from concourse.bass_utils import run_bass_kernel_spmd

B, T, D_STATE, D_MODEL, D_LATENT = 8, 4096, 1088, 1024, 128
N_CORES = 8
P = 128
F32 = mybir.dt.float32
BF16 = mybir.dt.bfloat16

# feature chunks of the contraction dim (1088 = 8*128 + 64)
R_CHUNKS = [(j * 128, min(128, D_STATE - j * 128)) for j in range((D_STATE + 127) // 128)]
NJ = len(R_CHUNKS)

_NC_CACHE = {}

SLABS = [256, 768, 1024, 1024, 1024]
KV_FIRST = 2      # leading slabs fully k/v-first (deferred stores)
KV2_GROUPS = 4    # first groups of slab 2 also k/v-first
LAT_SPLIT = 2048  # latent columns loaded in the first (small) piece


def build_nc(mm_dt=BF16, out_dt=BF16, in_bufs=3, wc_bufs=2, out_bufs=2,
             warmup_mms=12):
    """Build + compile the per-core Bass program (identical on all cores)."""
    assert sum(SLABS) == T
    max_sz = max(SLABS)

    nc = bacc.Bacc("TRN2", target_bir_lowering=False, debug=False, num_devices=N_CORES)

    # feature-major inputs: sT/wcT [1088, T], latT [128, T], wkvt [128,2,1024]
    st_d = nc.dram_tensor("st", [D_STATE, T], mm_dt, kind="ExternalInput").ap()
    wct_d = nc.dram_tensor("wct", [D_STATE, T], mm_dt, kind="ExternalInput").ap()
    latt_d = nc.dram_tensor("latt", [D_LATENT, T], mm_dt, kind="ExternalInput").ap()
    wrt_d = nc.dram_tensor("wrt", [D_STATE, D_MODEL], mm_dt, kind="ExternalInput").ap()
    wkvt_d = nc.dram_tensor("wkvt", [D_LATENT, 2, D_MODEL], mm_dt,
                            kind="ExternalInput").ap()
    out_d = nc.dram_tensor("out", [T, D_MODEL], out_dt, kind="ExternalOutput").ap()
    k_d = nc.dram_tensor("k", [T, D_MODEL], out_dt, kind="ExternalOutput").ap()
    v_d = nc.dram_tensor("v", [T, D_MODEL], out_dt, kind="ExternalOutput").ap()

    with tile.TileContext(nc) as tc:
        with (
            tc.tile_pool(name="weights", bufs=1) as wpool,
            tc.tile_pool(name="ins", bufs=in_bufs) as inpool,
            tc.tile_pool(name="wcp", bufs=wc_bufs) as wcpool,
            tc.tile_pool(name="outs", bufs=out_bufs) as outpool,
            tc.tile_pool(name="kv0", bufs=6) as kv0pool,
            tc.tile_pool(name="psum_out", bufs=2, space="PSUM") as pout_pool,
            tc.tile_pool(name="psum_kv", bufs=2, space="PSUM") as pkv_pool,
        ):
            # scalar queue: wkv + the first latent piece lead -> k/v matmuls
            # start ~9us in, while W_read/S/wc still stream
            wkv_t = wpool.tile([D_LATENT, 2, D_MODEL], mm_dt, tag="wkv")
            nc.scalar.dma_start(wkv_t[:], wkvt_d[:])
            lt = wpool.tile([D_LATENT, T], mm_dt, tag="lt")
            nc.scalar.dma_start(lt[:, 0:LAT_SPLIT], latt_d[:, 0:LAT_SPLIT])
            ltr = lt[:]

            def issue_loads(it):
                sz = SLABS[it]
                t0 = sum(SLABS[:it])
                xt = inpool.tile([P, NJ, sz], mm_dt, tag="x", name="xt",
                                 padded_shape=[P, NJ, max_sz])
                wc = wcpool.tile([P, NJ, sz], mm_dt, tag="wc", name="wc",
                                 padded_shape=[P, NJ, max_sz])
                nc.sync.dma_start(
                    xt[:, 0:8, :],
                    st_d[0:1024, t0 : t0 + sz].rearrange("(j p) t -> p j t", p=P))
                nc.sync.dma_start(xt[0:64, 8, :], st_d[1024:1088, t0 : t0 + sz])
                nc.scalar.dma_start(
                    wc[:, 0:8, :],
                    wct_d[0:1024, t0 : t0 + sz].rearrange("(j p) t -> p j t", p=P))
                nc.scalar.dma_start(wc[0:64, 8, :], wct_d[1024:1088, t0 : t0 + sz])
                return xt, wc

            def emit_adds(xt, wc, sz):
                xr = xt[:]
                for g in range(sz // P):
                    sl = slice(g * P, (g + 1) * P)
                    nc.vector.tensor_add(xr[:, :, sl], xt[:, :, sl], wc[:, :, sl])
                return xr

            # slab 0 inputs lead, then W_read split across BOTH queues so
            # each queue carries ~half of the out-critical ramp bytes
            slab_tiles = {0: issue_loads(0)}
            wr_all = wpool.tile([P, NJ, D_MODEL], mm_dt, tag="wr")
            nc.sync.dma_start(
                wr_all[:, 0:5, :],
                wrt_d[0:640, :].rearrange("(j p) n -> p j n", p=P))
            nc.scalar.dma_start(
                wr_all[:, 5:8, :],
                wrt_d[640:1024, :].rearrange("(j p) n -> p j n", p=P))
            nc.scalar.dma_start(wr_all[0:64, 8, :], wrt_d[1024:1088, :])
            slab_tiles[1] = issue_loads(1)
            nc.scalar.dma_start(lt[:, LAT_SPLIT:T], latt_d[:, LAT_SPLIT:T])

            # adds for slabs 0 and 1 (DVE picks them up as the loads land)
            xr0 = emit_adds(*slab_tiles[0], SLABS[0])
            xr1 = emit_adds(*slab_tiles[1], SLABS[1])
            xrs = {0: xr0, 1: xr1}

            # HAM warm-up: junk matmuls on a zeroed scratch tile keep the PE
            # busy through the clock-gate window while the first loads land
            if warmup_mms:
                scratch = wpool.tile([P, 512], mm_dt, tag="scratch")
                nc.vector.memset(scratch[:], 0.0)
                pj = pout_pool.tile([P, D_MODEL], F32, tag="pout", name="pjunk")
                for w in range(warmup_mms):
                    nc.tensor.matmul(pj[:, 0:512], scratch[:, 0:P],
                                     scratch[:, 0:512], start=True, stop=True)
                # keep the junk matmuls live
                keep = wpool.tile([1, 8], F32, tag="keep")
                nc.vector.tensor_copy(keep[:], pj[0:1, 0:8])

            def emit_kv(ts_abs, k_sb, v_sb, pool=None):
                pool = pool or pkv_pool
                tg = "pkv" if pool is pkv_pool else "pout"
                pk = pool.tile([P, D_MODEL], F32, tag=tg, name="pk")
                for h in range(2):
                    nc.tensor.matmul(
                        pk[:, h * 512 : h * 512 + 512],
                        ltr[:, ts_abs : ts_abs + P],
                        wkv_t[:, 0, h * 512 : h * 512 + 512],
                        start=True, stop=True)
                nc.scalar.copy(k_sb, pk[:])
                pv = pool.tile([P, D_MODEL], F32, tag=tg, name="pv")
                for h in range(2):
                    nc.tensor.matmul(
                        pv[:, h * 512 : h * 512 + 512],
                        ltr[:, ts_abs : ts_abs + P],
                        wkv_t[:, 1, h * 512 : h * 512 + 512],
                        start=True, stop=True)
                nc.vector.tensor_copy(v_sb, pv[:])

            def emit_out(xr, ts0, out_sb, copy_eng):
                po = pout_pool.tile([P, D_MODEL], F32, tag="pout", name="po")
                for j, (r0, rw) in enumerate(R_CHUNKS):
                    for h in range(2):
                        nc.tensor.matmul(
                            po[:, h * 512 : h * 512 + 512],
                            xr[0:rw, j, ts0 : ts0 + P],
                            wr_all[0:rw, j, h * 512 : h * 512 + 512],
                            start=(j == 0),
                            stop=(j == NJ - 1),
                        )
                if copy_eng is nc.scalar:
                    nc.scalar.copy(out_sb, po[:])
                else:
                    nc.vector.tensor_copy(out_sb, po[:])

            def pair_store(eng, dram, tl, row0):
                dst = dram[row0 : row0 + 2 * P, :].rearrange("(g p) d -> p g d", p=P)
                eng.dma_start(dst, tl[:])

            # ---- ramp: k/v for slabs 0-1 (stores deferred), then their out
            deferred_kv = []
            ramp_kv = [(it, SLABS[it] // P) for it in range(KV_FIRST)]
            ramp_kv.append((KV_FIRST, KV2_GROUPS))
            for it, ngr in ramp_kv:
                t0 = sum(SLABS[:it])
                kp = vp = None
                for g in range(ngr):
                    if g % 2 == 0:
                        kp = kv0pool.tile([P, 2, D_MODEL], out_dt, tag="k0",
                                          name="k0_pr")
                        vp = kv0pool.tile([P, 2, D_MODEL], out_dt, tag="v0",
                                          name="v0_pr")
                    emit_kv(t0 + g * P, kp[:, g % 2, :], vp[:, g % 2, :])
                    if g % 2 == 1:
                        deferred_kv.append((t0 + (g - 1) * P, kp, vp))

            def emit_out_slab(it):
                t0 = sum(SLABS[:it])
                opair = None
                for g in range(SLABS[it] // P):
                    if g % 2 == 0:
                        opair = outpool.tile([P, 2, D_MODEL], out_dt,
                                             tag="out", name="out_pr")
                    emit_out(xrs[it], g * P, opair[:, g % 2, :],
                             nc.vector if g % 2 == 0 else nc.scalar)
                    if g % 2 == 1:
                        pair_store(nc.scalar if (g // 2) % 2 == 0 else nc.sync,
                                   out_d, opair, t0 + (g - 1) * P)

            emit_out_slab(0)

            # ---- steady slabs ----
            for it in range(1, len(SLABS)):
                sz = SLABS[it]
                t0 = sum(SLABS[:it])

                # prefetch + adds for the NEXT slab before this slab's stores
                if it + 1 < len(SLABS):
                    nxt = issue_loads(it + 1)
                    slab_tiles[it + 1] = nxt
                    xrs[it + 1] = emit_adds(*nxt, SLABS[it + 1])

                # flush deferred ramp k/v stores across slabs 2-3
                if it >= KV_FIRST and deferred_kv:
                    nflush = 2 if it < len(SLABS) - 1 else len(deferred_kv)
                    # (6 ramp pairs total: 2 at it=2, 2 at it=3, rest at it=4)
                    for row0, kp, vp in deferred_kv[:nflush]:
                        pair_store(nc.scalar, k_d, kp, row0)
                        pair_store(nc.sync, v_d, vp, row0)
                    deferred_kv = deferred_kv[nflush:]

                if it < KV_FIRST:
                    emit_out_slab(it)
                    continue

                xr = xrs[it]
                tiles = None
                for g in range(sz // P):
                    if g % 2 == 0:
                        need_kv = not (it == KV_FIRST and g + 1 < KV2_GROUPS)
                        tiles = (
                            outpool.tile([P, 2, D_MODEL], out_dt, tag="out",
                                         name="out_pr"),
                            outpool.tile([P, 2, D_MODEL], out_dt, tag="k",
                                         name="k_pr") if need_kv else None,
                            outpool.tile([P, 2, D_MODEL], out_dt, tag="v",
                                         name="v_pr") if need_kv else None,
                        )
                    kv_inline = not (it == KV_FIRST and g < KV2_GROUPS)
                    last_pair = (it == len(SLABS) - 1) and g >= sz // P - 2
                    if last_pair and g == sz // P - 1:
                        # tail: k/v first so the final PE burst is the out
                        # GEMM and its single copy+store close the kernel
                        emit_kv(t0 + g * P, tiles[1][:, g % 2, :],
                                tiles[2][:, g % 2, :])
                        emit_out(xr, g * P, tiles[0][:, g % 2, :], nc.scalar)
                    else:
                        emit_out(xr, g * P, tiles[0][:, g % 2, :],
                                 nc.vector if g % 2 == 0 else nc.scalar)
                        if kv_inline:
                            emit_kv(t0 + g * P, tiles[1][:, g % 2, :],
                                    tiles[2][:, g % 2, :])
                    if last_pair:
                        # tail: store each final group immediately (unpaired)
                        row0 = t0 + g * P
                        eng = [nc.scalar, nc.sync] if g % 2 == 0 else \
                              [nc.sync, nc.scalar]
                        eng[0].dma_start(out_d[row0 : row0 + P, :],
                                         tiles[0][:, g % 2, :])
                        eng[1].dma_start(k_d[row0 : row0 + P, :],
                                         tiles[1][:, g % 2, :])
                        eng[0].dma_start(v_d[row0 : row0 + P, :],
                                         tiles[2][:, g % 2, :])
                    elif g % 2 == 1:
                        row0 = t0 + (g - 1) * P
                        eng = [nc.scalar, nc.sync] if (g // 2) % 2 == 0 else \
                              [nc.sync, nc.scalar]
                        pair_store(eng[0], out_d, tiles[0], row0)
                        if kv_inline:
                            pair_store(eng[1], k_d, tiles[1], row0)
                            pair_store(eng[0], v_d, tiles[2], row0)

    nc.compile()
    return nc


def _get_nc(**kw):
    key = tuple(sorted(kw.items()))
    if key not in _NC_CACHE:
        _NC_CACHE[key] = build_nc(**kw)
    return _NC_CACHE[key]


def make_in_maps(S, w_spoke, w_hub_priv, w_hub_shared, tag, W_read, cache, latent,
                 Wk, Wv):
    # host-side layout prep only (shard over batch, feature-major transposes,
    # bf16 downcast)
    bf = ml_dtypes.bfloat16
    wcat = np.concatenate(
        [np.asarray(w_spoke, np.float32), np.asarray(w_hub_priv, np.float32),
         np.asarray(w_hub_shared, np.float32), np.asarray(tag, np.float32)],
        axis=-1,
    )
    sT = np.ascontiguousarray(np.asarray(S, np.float32).transpose(0, 2, 1)).astype(bf)
    wcT = np.ascontiguousarray(wcat.transpose(0, 2, 1)).astype(bf)
    latT = np.ascontiguousarray(
        np.asarray(latent, np.float32).transpose(0, 2, 1)).astype(bf)
    wrt = np.ascontiguousarray(np.asarray(W_read, np.float32).T).astype(bf)
    wkvt = np.ascontiguousarray(
        np.stack([np.asarray(Wk, np.float32).T, np.asarray(Wv, np.float32).T],
                 axis=1)).astype(bf)
    return [
        {"st": sT[i], "wct": wcT[i], "latt": latT[i], "wrt": wrt, "wkvt": wkvt}
        for i in range(N_CORES)
    ]


def kernel(S, w_spoke, w_hub_priv, w_hub_shared, tag, W_read, cache, latent, Wk, Wv,
           **build_kw):
    in_maps = make_in_maps(S, w_spoke, w_hub_priv, w_hub_shared, tag, W_read, cache,
                           latent, Wk, Wv)
    nc = _get_nc(**build_kw)
    res = run_bass_kernel_spmd(nc, in_maps, list(range(N_CORES)))
    out = np.stack([res.results[i]["out"].astype(np.float32) for i in range(N_CORES)])
    k = np.stack([res.results[i]["k"].astype(np.float32) for i in range(N_CORES)])
    v = np.stack([res.results[i]["v"].astype(np.float32) for i in range(N_CORES)])
    return (out, k, v)


# revision 17
# speedup vs baseline: 1.0433x; 1.0433x over previous
"""Trainium2 Bass kernel for nn_CognitiveWorkspaceTransformer.

Math (reference semantics):
    X   = S + concat(w_spoke, w_hub_priv, w_hub_shared, tag)   # full 1088 cover
    out = X @ W_read.T          # (B,T,1024)
    k   = latent @ Wk.T         # cache is fully overwritten by latent
    v   = latent @ Wv.T

Sharding: data-parallel over batch B=8, one batch element per NeuronCore.
All tensors are laid out feature-major on the host (pure layout prep plus a
bf16 downcast, no arithmetic) so the contraction dim lands on SBUF
partitions directly and the PE needs no on-chip transposes.

bf16 everywhere (tolerance is 2e-2; bf16 lands ~5e-3): ~47MB/core HBM
traffic (~131us roofline) vs ~360k PE cycles (~150us @ 2.4GHz) -> the PE
array is the bottleneck; everything else is scheduled to keep it fed:
  - a few warm-up junk matmuls at t=0 so the HAM clock gate reaches
    2.4GHz before real work lands (cold matmuls run at 1.2GHz)
  - the ramp is ordered for earliest PE start: Wk/Wv (0.5MB) and the
    first 768 latent columns load first, then slab-0/1 S+wc, then the
    bulky W_read; slabs 0-1 are small (256/512) and run their k/v
    matmuls before the first out-matmul; their k/v stores are deferred
    to slabs 2-3 so ramp loads keep the full HBM bandwidth
  - adds are emitted one slab ahead so they never compete with a slab
    tail's PSUM->SBUF copies on the DVE (pout ring would stall the PE)
  - out-copies alternate DVE/ACT per group pair for the same reason
  - slab i+1 loads are issued BEFORE slab i stores: a store waiting on
    its tile would otherwise block later load issues (in-order queues)
  - j-outer/h-inner so each 128x128 stationary X^T chunk is loaded once
  - 2-bank PSUM tiles [128,1024]; ONE wide PSUM->SBUF cast-copy per
    out/k/v tile; paired [256,1024] stores
"""

import numpy as np
import ml_dtypes

import concourse.bacc as bacc
import concourse.mybir as mybir
import concourse.tile as tile
from concourse.bass_utils import run_bass_kernel_spmd

B, T, D_STATE, D_MODEL, D_LATENT = 8, 4096, 1088, 1024, 128
N_CORES = 8
P = 128
F32 = mybir.dt.float32
BF16 = mybir.dt.bfloat16

# feature chunks of the contraction dim (1088 = 8*128 + 64)
R_CHUNKS = [(j * 128, min(128, D_STATE - j * 128)) for j in range((D_STATE + 127) // 128)]
NJ = len(R_CHUNKS)

_NC_CACHE = {}

SLABS = [256, 768, 1024, 1024, 1024]
KV_FIRST = 2      # leading slabs fully k/v-first (deferred stores)
KV2_GROUPS = 0    # first groups of slab 2 also k/v-first
LAT_SPLIT = 1024  # latent columns loaded in the first (small) piece


def build_nc(mm_dt=BF16, out_dt=BF16, in_bufs=3, wc_bufs=2, out_bufs=2,
             warmup_mms=12):
    """Build + compile the per-core Bass program (identical on all cores)."""
    assert sum(SLABS) == T
    max_sz = max(SLABS)

    nc = bacc.Bacc("TRN2", target_bir_lowering=False, debug=False, num_devices=N_CORES)

    # feature-major inputs: sT/wcT [1088, T], latT [128, T], wkvt [128,2,1024]
    st_d = nc.dram_tensor("st", [D_STATE, T], mm_dt, kind="ExternalInput").ap()
    wct_d = nc.dram_tensor("wct", [D_STATE, T], mm_dt, kind="ExternalInput").ap()
    latt_d = nc.dram_tensor("latt", [D_LATENT, T], mm_dt, kind="ExternalInput").ap()
    wrt_d = nc.dram_tensor("wrt", [D_STATE, D_MODEL], mm_dt, kind="ExternalInput").ap()
    wkvt_d = nc.dram_tensor("wkvt", [D_LATENT, 2, D_MODEL], mm_dt,
                            kind="ExternalInput").ap()
    out_d = nc.dram_tensor("out", [T, D_MODEL], out_dt, kind="ExternalOutput").ap()
    k_d = nc.dram_tensor("k", [T, D_MODEL], out_dt, kind="ExternalOutput").ap()
    v_d = nc.dram_tensor("v", [T, D_MODEL], out_dt, kind="ExternalOutput").ap()

    with tile.TileContext(nc) as tc:
        with (
            tc.tile_pool(name="weights", bufs=1) as wpool,
            tc.tile_pool(name="ins", bufs=in_bufs) as inpool,
            tc.tile_pool(name="wcp", bufs=wc_bufs) as wcpool,
            tc.tile_pool(name="outs", bufs=out_bufs) as outpool,
            tc.tile_pool(name="kv0", bufs=4) as kv0pool,
            tc.tile_pool(name="psum_out", bufs=2, space="PSUM") as pout_pool,
            tc.tile_pool(name="psum_kv", bufs=2, space="PSUM") as pkv_pool,
        ):
            # scalar queue: wkv + the first latent piece lead -> k/v matmuls
            # start ~9us in, while W_read/S/wc still stream
            wkv_t = wpool.tile([D_LATENT, 2, D_MODEL], mm_dt, tag="wkv")
            nc.scalar.dma_start(wkv_t[:], wkvt_d[:])
            lt = wpool.tile([D_LATENT, T], mm_dt, tag="lt")
            nc.scalar.dma_start(lt[:, 0:LAT_SPLIT], latt_d[:, 0:LAT_SPLIT])
            ltr = lt[:]

            def issue_loads(it):
                sz = SLABS[it]
                t0 = sum(SLABS[:it])
                xt = inpool.tile([P, NJ, sz], mm_dt, tag="x", name="xt",
                                 padded_shape=[P, NJ, max_sz])
                wc = wcpool.tile([P, NJ, sz], mm_dt, tag="wc", name="wc",
                                 padded_shape=[P, NJ, max_sz])
                nc.sync.dma_start(
                    xt[:, 0:8, :],
                    st_d[0:1024, t0 : t0 + sz].rearrange("(j p) t -> p j t", p=P))
                nc.sync.dma_start(xt[0:64, 8, :], st_d[1024:1088, t0 : t0 + sz])
                nc.scalar.dma_start(
                    wc[:, 0:8, :],
                    wct_d[0:1024, t0 : t0 + sz].rearrange("(j p) t -> p j t", p=P))
                nc.scalar.dma_start(wc[0:64, 8, :], wct_d[1024:1088, t0 : t0 + sz])
                return xt, wc

            def emit_adds(xt, wc, sz):
                xr = xt[:]
                for g in range(sz // P):
                    sl = slice(g * P, (g + 1) * P)
                    nc.vector.tensor_add(xr[:, :, sl], xt[:, :, sl], wc[:, :, sl])
                return xr

            # slab 0 inputs lead, then W_read split across BOTH queues so
            # each queue carries ~half of the out-critical ramp bytes
            slab_tiles = {0: issue_loads(0)}
            wr_all = wpool.tile([P, NJ, D_MODEL], mm_dt, tag="wr")
            nc.sync.dma_start(
                wr_all[:, 0:5, :],
                wrt_d[0:640, :].rearrange("(j p) n -> p j n", p=P))
            nc.scalar.dma_start(
                wr_all[:, 5:8, :],
                wrt_d[640:1024, :].rearrange("(j p) n -> p j n", p=P))
            nc.scalar.dma_start(wr_all[0:64, 8, :], wrt_d[1024:1088, :])
            slab_tiles[1] = issue_loads(1)
            nc.scalar.dma_start(lt[:, LAT_SPLIT:T], latt_d[:, LAT_SPLIT:T])

            # adds for slabs 0 and 1 (DVE picks them up as the loads land)
            xr0 = emit_adds(*slab_tiles[0], SLABS[0])
            xr1 = emit_adds(*slab_tiles[1], SLABS[1])
            xrs = {0: xr0, 1: xr1}

            # HAM warm-up + gap fillers: junk matmuls on a zeroed scratch
            # tile keep the PE clock-gate at 2.4GHz through load waits (an
            # idle window >3.4us halves the PE clock for the next ~4-7us)
            scratch = wpool.tile([P, 512], mm_dt, tag="scratch")
            nc.vector.memset(scratch[:], 0.0)
            keep = wpool.tile([1, 8], F32, tag="keep")

            def junk_fill(n):
                if n <= 0:
                    return
                pj = pout_pool.tile([P, D_MODEL], F32, tag="pout", name="pjunk")
                for w in range(n):
                    nc.tensor.matmul(pj[:, 0:512], scratch[:, 0:P],
                                     scratch[:, 0:512], start=True, stop=True)
                # keep the junk matmuls live
                nc.vector.tensor_copy(keep[:], pj[0:1, 0:8])

            junk_fill(warmup_mms)

            def emit_kv(ts_abs, k_sb, v_sb, pool=None):
                pool = pool or pkv_pool
                tg = "pkv" if pool is pkv_pool else "pout"
                pk = pool.tile([P, D_MODEL], F32, tag=tg, name="pk")
                for h in range(2):
                    nc.tensor.matmul(
                        pk[:, h * 512 : h * 512 + 512],
                        ltr[:, ts_abs : ts_abs + P],
                        wkv_t[:, 0, h * 512 : h * 512 + 512],
                        start=True, stop=True)
                nc.scalar.copy(k_sb, pk[:])
                pv = pool.tile([P, D_MODEL], F32, tag=tg, name="pv")
                for h in range(2):
                    nc.tensor.matmul(
                        pv[:, h * 512 : h * 512 + 512],
                        ltr[:, ts_abs : ts_abs + P],
                        wkv_t[:, 1, h * 512 : h * 512 + 512],
                        start=True, stop=True)
                nc.vector.tensor_copy(v_sb, pv[:])

            def emit_out(xr, ts0, out_sb, copy_eng):
                po = pout_pool.tile([P, D_MODEL], F32, tag="pout", name="po")
                for j, (r0, rw) in enumerate(R_CHUNKS):
                    for h in range(2):
                        nc.tensor.matmul(
                            po[:, h * 512 : h * 512 + 512],
                            xr[0:rw, j, ts0 : ts0 + P],
                            wr_all[0:rw, j, h * 512 : h * 512 + 512],
                            start=(j == 0),
                            stop=(j == NJ - 1),
                        )
                if copy_eng is nc.scalar:
                    nc.scalar.copy(out_sb, po[:])
                else:
                    nc.vector.tensor_copy(out_sb, po[:])

            def pair_store(eng, dram, tl, row0):
                dst = dram[row0 : row0 + 2 * P, :].rearrange("(g p) d -> p g d", p=P)
                eng.dma_start(dst, tl[:])

            # ---- ramp: k/v for slabs 0-1 (stores deferred), then their out
            deferred_kv = []
            ramp_kv = [(it, SLABS[it] // P) for it in range(KV_FIRST)]
            ramp_kv.append((KV_FIRST, KV2_GROUPS))
            for it, ngr in ramp_kv:
                t0 = sum(SLABS[:it])
                kp = vp = None
                for g in range(ngr):
                    if g % 2 == 0:
                        kp = kv0pool.tile([P, 2, D_MODEL], out_dt, tag="k0",
                                          name="k0_pr")
                        vp = kv0pool.tile([P, 2, D_MODEL], out_dt, tag="v0",
                                          name="v0_pr")
                    emit_kv(t0 + g * P, kp[:, g % 2, :], vp[:, g % 2, :])
                    if g % 2 == 1:
                        deferred_kv.append((t0 + (g - 1) * P, kp, vp))

            def emit_out_slab(it):
                t0 = sum(SLABS[:it])
                opair = None
                for g in range(SLABS[it] // P):
                    if g % 2 == 0:
                        opair = outpool.tile([P, 2, D_MODEL], out_dt,
                                             tag="out", name="out_pr")
                    emit_out(xrs[it], g * P, opair[:, g % 2, :],
                             nc.vector if g % 2 == 0 else nc.scalar)
                    if g % 2 == 1:
                        pair_store(nc.scalar if (g // 2) % 2 == 0 else nc.sync,
                                   out_d, opair, t0 + (g - 1) * P)

            junk_fill(10)
            emit_out_slab(0)
            junk_fill(20)

            # ---- steady slabs ----
            for it in range(1, len(SLABS)):
                sz = SLABS[it]
                t0 = sum(SLABS[:it])

                # prefetch + adds for the NEXT slab before this slab's stores
                if it + 1 < len(SLABS):
                    nxt = issue_loads(it + 1)
                    slab_tiles[it + 1] = nxt
                    xrs[it + 1] = emit_adds(*nxt, SLABS[it + 1])

                # flush deferred ramp k/v stores across slabs 2-3
                if it >= KV_FIRST and deferred_kv:
                    nflush = 2 if it < len(SLABS) - 1 else len(deferred_kv)
                    # (6 ramp pairs total: 2 at it=2, 2 at it=3, rest at it=4)
                    for row0, kp, vp in deferred_kv[:nflush]:
                        pair_store(nc.scalar, k_d, kp, row0)
                        pair_store(nc.sync, v_d, vp, row0)
                    deferred_kv = deferred_kv[nflush:]

                if it < KV_FIRST:
                    emit_out_slab(it)
                    continue

                xr = xrs[it]
                tiles = None
                for g in range(sz // P):
                    if g % 2 == 0:
                        need_kv = not (it == KV_FIRST and g + 1 < KV2_GROUPS)
                        tiles = (
                            outpool.tile([P, 2, D_MODEL], out_dt, tag="out",
                                         name="out_pr"),
                            outpool.tile([P, 2, D_MODEL], out_dt, tag="k",
                                         name="k_pr") if need_kv else None,
                            outpool.tile([P, 2, D_MODEL], out_dt, tag="v",
                                         name="v_pr") if need_kv else None,
                        )
                    kv_inline = not (it == KV_FIRST and g < KV2_GROUPS)
                    last_pair = (it == len(SLABS) - 1) and g >= sz // P - 2
                    if last_pair and g == sz // P - 1:
                        # tail: k/v first so the final PE burst is the out
                        # GEMM and its single copy+store close the kernel
                        emit_kv(t0 + g * P, tiles[1][:, g % 2, :],
                                tiles[2][:, g % 2, :])
                        emit_out(xr, g * P, tiles[0][:, g % 2, :], nc.scalar)
                    else:
                        emit_out(xr, g * P, tiles[0][:, g % 2, :],
                                 nc.vector if g % 2 == 0 else nc.scalar)
                        if kv_inline:
                            emit_kv(t0 + g * P, tiles[1][:, g % 2, :],
                                    tiles[2][:, g % 2, :])
                    if last_pair:
                        # tail: store each final group immediately (unpaired)
                        row0 = t0 + g * P
                        eng = [nc.scalar, nc.sync] if g % 2 == 0 else \
                              [nc.sync, nc.scalar]
                        eng[0].dma_start(out_d[row0 : row0 + P, :],
                                         tiles[0][:, g % 2, :])
                        eng[1].dma_start(k_d[row0 : row0 + P, :],
                                         tiles[1][:, g % 2, :])
                        eng[0].dma_start(v_d[row0 : row0 + P, :],
                                         tiles[2][:, g % 2, :])
                    elif g % 2 == 1:
                        row0 = t0 + (g - 1) * P
                        eng = [nc.scalar, nc.sync] if (g // 2) % 2 == 0 else \
                              [nc.sync, nc.scalar]
                        pair_store(eng[0], out_d, tiles[0], row0)
                        if kv_inline:
                            pair_store(eng[1], k_d, tiles[1], row0)
                            pair_store(eng[0], v_d, tiles[2], row0)

    nc.compile()
    return nc


def _get_nc(**kw):
    key = tuple(sorted(kw.items()))
    if key not in _NC_CACHE:
        _NC_CACHE[key] = build_nc(**kw)
    return _NC_CACHE[key]


def make_in_maps(S, w_spoke, w_hub_priv, w_hub_shared, tag, W_read, cache, latent,
                 Wk, Wv):
    # host-side layout prep only (shard over batch, feature-major transposes,
    # bf16 downcast)
    bf = ml_dtypes.bfloat16
    wcat = np.concatenate(
        [np.asarray(w_spoke, np.float32), np.asarray(w_hub_priv, np.float32),
         np.asarray(w_hub_shared, np.float32), np.asarray(tag, np.float32)],
        axis=-1,
    )
    sT = np.ascontiguousarray(np.asarray(S, np.float32).transpose(0, 2, 1)).astype(bf)
    wcT = np.ascontiguousarray(wcat.transpose(0, 2, 1)).astype(bf)
    latT = np.ascontiguousarray(
        np.asarray(latent, np.float32).transpose(0, 2, 1)).astype(bf)
    wrt = np.ascontiguousarray(np.asarray(W_read, np.float32).T).astype(bf)
    wkvt = np.ascontiguousarray(
        np.stack([np.asarray(Wk, np.float32).T, np.asarray(Wv, np.float32).T],
                 axis=1)).astype(bf)
    return [
        {"st": sT[i], "wct": wcT[i], "latt": latT[i], "wrt": wrt, "wkvt": wkvt}
        for i in range(N_CORES)
    ]


def kernel(S, w_spoke, w_hub_priv, w_hub_shared, tag, W_read, cache, latent, Wk, Wv,
           **build_kw):
    in_maps = make_in_maps(S, w_spoke, w_hub_priv, w_hub_shared, tag, W_read, cache,
                           latent, Wk, Wv)
    nc = _get_nc(**build_kw)
    res = run_bass_kernel_spmd(nc, in_maps, list(range(N_CORES)))
    out = np.stack([res.results[i]["out"].astype(np.float32) for i in range(N_CORES)])
    k = np.stack([res.results[i]["k"].astype(np.float32) for i in range(N_CORES)])
    v = np.stack([res.results[i]["v"].astype(np.float32) for i in range(N_CORES)])
    return (out, k, v)
